# revision 19
# baseline (speedup 1.0000x reference)
"""Trainium2 Bass kernel for nn_DSTQFunction_28415503630466.

Math: the reference augments each 2-point/2-channel sequence with a pointwise
linear layer, concatenates to a 4-channel 2-point path, takes its depth-4
path signature (340 features), appends seq[:,:,-1], and applies a 2-layer MLP.
Every feature is a polynomial of degree <= 4 in the 4 raw inputs z, so the
whole pre-relu layer folds into h = A @ mono(z) + b1' where mono(z) are the 69
non-constant monomials of degree <= 4 (A computed host-side in float64).

Device (pure data parallel, 8 cores, per core B=32768 = 128 chunks x 256):
  - z arrives host-packed [128 chunks, 4*256] f32 (chunk-major, feature blocks)
  - DVE: 12 broadcast-AP tensor_mul ops build all 65 product monomials in fp16
  - DMA: relayout monomial blocks into rhs [feature-row, chunk*256+e] fp16,
    feature rows spread over partitions 0-34 / 64-97 to engage all DMA ports
  - PE: h = A.T(stationary) @ rhs per chunk, two column-packed copies of A
    (array cols 0-63 / 64-127) write PSUM rows 0-63 / 64-127 alternately
  - ACT: relu(h + b1) PSUM->SBUF fp16
  - PE: block-diag [W2;W2] (K=128) contracts pairs of chunks -> [6, N] PSUM
  - DVE: scalar_tensor_tensor adds b2 and exits PSUM->SBUF f32
  - DMA out; host reassembles (B, 3).
"""
import os
import sys

for _p in ("/opt/trn_rl_repo", "/root/.axon_site/_ro/trn_rl_repo"):
    if os.path.isdir(_p) and _p not in sys.path:
        sys.path.insert(0, _p)

import numpy as np
import concourse.bacc as bacc
import concourse.mybir as mybir
import concourse.tile as tile
from concourse.bass_utils import run_bass_kernel_spmd

F16 = mybir.dt.float16
F32 = mybir.dt.float32

N_CORES = 8
NE = 256      # elements per chunk
NCHUNK = 128  # chunks per core
B_CORE = NE * NCHUNK
N_SEGS = 4

# ---------------- host-side polynomial algebra ----------------
DEG2 = [(i, j) for i in range(4) for j in range(i, 4)]            # 10
DEG3 = [(i,) + p for i in range(4) for p in DEG2 if p[0] >= i]    # 20
DEG4 = [(i,) + p for i in range(4) for p in DEG3 if p[0] >= i]    # 35
MONOS = [(i,) for i in range(4)] + DEG2 + DEG3 + DEG4             # 69
MONO_INDEX = {m: k for k, m in enumerate(MONOS)}
NF = 69

D2_LEN = [4, 3, 2, 1]
D2_START = [0, 4, 7, 9]
D3_LEN = [10, 6, 3, 1]
D3_START = [0, 10, 16, 19]
D4_LEN = [20, 10, 4, 1]
D4_START = [0, 20, 30, 34]
D2_OFF, D3_OFF, D4_OFF = 0, 10, 30
K_SPAN = 69  # contiguous feature rows: z(0-3) deg2(4-13) deg3(14-33) deg4(34-68)


def _poly_add(a, b, sb=1.0):
    out = dict(a)
    for k, v in b.items():
        out[k] = out.get(k, 0.0) + sb * v
    return out


def _poly_scale(a, s):
    return {k: v * s for k, v in a.items()}


def _poly_mul(a, b):
    out = {}
    for ka, va in a.items():
        for kb, vb in b.items():
            k = tuple(sorted(ka + kb))
            out[k] = out.get(k, 0.0) + va * vb
    return out


def _build_A(W_aug, b_aug, W1, b1):
    """Fold augment + depth-4 signature (Chen) + W1 into (A (64,69), bias (64,))."""
    W_aug = np.asarray(W_aug, np.float64)
    b_aug = np.asarray(b_aug, np.float64)
    W1 = np.asarray(W1, np.float64)
    b1 = np.asarray(b1, np.float64)
    z = [{(i,): 1.0} for i in range(4)]

    def aug(l, d):
        s0, s1 = z[2 * l + 0], z[2 * l + 1]
        out = _poly_add(_poly_scale(s0, W_aug[d, 0]), _poly_scale(s1, W_aug[d, 1]))
        return _poly_add(out, {(): b_aug[d]})

    p = [[z[2 * l], z[2 * l + 1], aug(l, 0), aug(l, 1)] for l in range(2)]
    u = p[0]
    v = [_poly_add(p[1][c], p[0][c], -1.0) for c in range(4)]

    def sig_exp(dx):
        levels = [[dx[c] for c in range(4)]]
        for k in range(2, 5):
            levels.append(
                [_poly_scale(_poly_mul(a, dx[c]), 1.0 / k) for a in levels[-1] for c in range(4)]
            )
        return levels

    A_lv, B_lv = sig_exp(u), sig_exp(v)
    C = []
    for k in range(1, 5):
        c = [_poly_add(x, y) for x, y in zip(A_lv[k - 1], B_lv[k - 1])]
        for i in range(1, k):
            o = [_poly_mul(x, y) for x in A_lv[i - 1] for y in B_lv[k - i - 1]]
            c = [_poly_add(x, y) for x, y in zip(c, o)]
        C.append(c)
    feats = [pp for lv in C for pp in lv] + [z[1], z[3]]
    T = np.zeros((342, NF + 1))
    for r, p_ in enumerate(feats):
        for k, vv in p_.items():
            if len(k) == 0:
                T[r, NF] += vv
            else:
                T[r, MONO_INDEX[k]] += vv
    A_full = W1 @ T
    return A_full[:, :NF], A_full[:, NF] + b1


def _row_of_feature(F):
    return F


def _build_consts(W_aug, b_aug, W1, b1, W2, b2):
    A, bias1 = _build_A(W_aug, b_aug, W1, b1)
    W2 = np.asarray(W2, np.float64)
    b2 = np.asarray(b2, np.float64)
    A_dev = np.zeros((128, 64), np.float16)
    for F in range(NF):
        A_dev[_row_of_feature(F), :] = A[:, F].astype(np.float16)
    w2blk = np.zeros((128, 32), np.float16)
    w2blk[0:64, 0:3] = W2.T
    w2blk[64:128, 3:6] = W2.T
    b1t = np.zeros((128, 1), np.float32)
    b1t[0:64, 0] = bias1
    b1t[64:128, 0] = bias1
    b2t = np.zeros((128, 1), np.float32)
    for p in range(128):
        if p % 32 < 6:
            b2t[p, 0] = b2[(p % 32) % 3]
    return A_dev, w2blk, b1t, b2t


def _host_pack_z(seq_core):
    return np.ascontiguousarray(
        seq_core.reshape(NCHUNK, NE, 4).transpose(0, 2, 1)
    ).reshape(NCHUNK, 4 * NE).astype(np.float16)


def _host_unpack_out(out_d):
    """Invert the on-chip layout chain:
    out_d[bi, r, f]: f = G*4NE + hh*2NE + u -> mm2 slice q = G*8 + hh*4 + bi,
    hrelu index H = q*512 + u -> h-group g = H//(4NE), w = H%(4NE) ->
    mm1 t = 2*(w//(2NE)) + (r//3), chunk-pair cp = g*4 + t, j = w%(2NE) ->
    chunk = 2cp + j//NE, e = j%NE."""
    res = np.empty((B_CORE, 3), np.float32)
    f = np.arange(16 * NE)
    G = f // (4 * NE)
    hh = (f % (4 * NE)) // (2 * NE)
    u = f % (2 * NE)
    for bi in range(4):
        q = G * 8 + hh * 4 + bi
        H = q * (2 * NE) + u
        g = H // (4 * NE)
        w = H % (4 * NE)
        j = w % (2 * NE)
        for r in range(6):
            t = 2 * (w // (2 * NE)) + (r // 3)
            cp = g * 4 + t
            chunk = 2 * cp + j // NE
            e = j % NE
            res[chunk * NE + e, r % 3] = out_d[bi, r, :]
    return res


# ---------------- device program ----------------
def _build_nc():
    nc = bacc.Bacc(target_bir_lowering=False)
    z_d = nc.dram_tensor("z_d", [NCHUNK, 4 * NE], F16, kind="ExternalInput")
    a_d = nc.dram_tensor("a_d", [128, 64], F16, kind="ExternalInput")
    w2_d = nc.dram_tensor("w2_d", [128, 32], F16, kind="ExternalInput")
    b1_d = nc.dram_tensor("b1_d", [128, 1], F32, kind="ExternalInput")
    b2_d = nc.dram_tensor("b2_d", [128, 1], F32, kind="ExternalInput")
    out_d = nc.dram_tensor("out_d", [4, 6, 16 * NE], F32, kind="ExternalOutput")

    seg_ch = NCHUNK // N_SEGS
    with tile.TileContext(nc) as tc:
        with (
            tc.tile_pool(name="consts", bufs=1) as pc,
            tc.tile_pool(name="zp", bufs=1) as pz,
            tc.tile_pool(name="monop", bufs=1) as pm,
            tc.tile_pool(name="rhsp", bufs=1) as pr,
            tc.tile_pool(name="hrelup", bufs=1) as ph,
            tc.tile_pool(name="outp", bufs=1) as po,
            tc.tile_pool(name="psh", bufs=2, space="PSUM") as psh,
            tc.tile_pool(name="pso", bufs=2, space="PSUM") as pso,
        ):
            a_t = pc.tile([128, 64], F16)
            w2_t = pc.tile([128, 32], F16)
            b1_t = pc.tile([128, 1], F32)
            b2_t = pc.tile([128, 1], F32)
            nc.scalar.dma_start(out=a_t[:], in_=a_d[:])
            nc.scalar.dma_start(out=w2_t[:], in_=w2_d[:])
            nc.scalar.dma_start(out=b1_t[:], in_=b1_d[:])
            nc.scalar.dma_start(out=b2_t[:], in_=b2_d[:])

            zcol = pc.tile([128, 1], F32)
            nc.vector.memset(zcol[:, :], 0.0)
            feat = pm.tile([NCHUNK, 69 * NE], F16)   # blocks: z(4) deg2(10) deg3(20) deg4(35)
            zh = feat[:, 0:4 * NE]
            rhs = pr.tile([128, NCHUNK * NE], F16)
            hrelu = ph.tile([128, (NCHUNK // 2) * NE], F16)
            outsb = po.tile([128, 16 * NE], F32)

            def blk(t, b0, n):
                return t[:, b0 * NE:(b0 + n) * NE].rearrange("p (f e) -> p f e", e=NE)

            ring = [nc.sync, nc.scalar]

            def relayout_rows(F0, n):
                # one row DMA per feature (128 x 512B descriptors each),
                # alternating the two HWDGE rings
                for F in range(F0, F0 + n):
                    ring[F % 2].dma_start(
                        out=rhs[F:F + 1, :], in_=feat[:, F * NE:(F + 1) * NE]
                    )

            # products: full 128-partition ops (DVE time = free-size; never
            # partition-slice); feat blocks: z at 0..3, deg-k groups offset +4
            nc.sync.dma_start(out=zh[:, :], in_=z_d[:, :])
            for i in range(4):
                n = D2_LEN[i]
                nc.vector.tensor_mul(
                    out=blk(feat, 4 + D2_OFF + D2_START[i], n),
                    in0=zh[:, i * NE:(i + 1) * NE].unsqueeze(1).broadcast_to([NCHUNK, n, NE]),
                    in1=blk(feat, i, n),
                )
            relayout_rows(0, 14)    # z + deg2 rows
            for i in (3, 2, 1, 0):
                n = D3_LEN[i]
                nc.vector.tensor_mul(
                    out=blk(feat, 4 + D3_OFF + D3_START[i], n),
                    in0=zh[:, i * NE:(i + 1) * NE].unsqueeze(1).broadcast_to([NCHUNK, n, NE]),
                    in1=blk(feat, 4 + D2_OFF + D2_START[i], n),
                )
            relayout_rows(14, 20)   # deg3 rows
            for i in (3, 2, 1, 0):
                n4 = D4_LEN[i]
                nc.vector.tensor_mul(
                    out=blk(feat, 4 + D4_OFF + D4_START[i], n4),
                    in0=zh[:, i * NE:(i + 1) * NE].unsqueeze(1).broadcast_to([NCHUNK, n4, NE]),
                    in1=blk(feat, 4 + D3_OFF + D3_START[i], n4),
                )
            relayout_rows(34, 35)   # deg4 rows

            for g in range(NCHUNK // 8):
                h = psh.tile([128, 4 * NE], F32)
                for t in range(4):  # chunk-pair (2t, 2t+1) -> one N=512 matmul
                    cp = g * 4 + t
                    half = slice(0, 64) if (t % 2 == 0) else slice(64, 128)
                    fr = (t // 2) * 2 * NE
                    nc.tensor.matmul(
                        out=h[half, fr:fr + 2 * NE],
                        lhsT=a_t[0:K_SPAN, :],
                        rhs=rhs[0:K_SPAN, cp * 2 * NE:(cp + 1) * 2 * NE],
                        start=True, stop=True,
                    )
                if g % 8 >= 3:  # balance: 10 groups on ACT (late), 6 on DVE (early)
                    nc.scalar.activation(
                        out=hrelu[:, g * 4 * NE:(g + 1) * 4 * NE],
                        in_=h[:, :],
                        func=mybir.ActivationFunctionType.Relu,
                        bias=b1_t[:, 0:1],
                        scale=1.0,
                    )
                else:
                    nc.vector.scalar_tensor_tensor(
                        out=hrelu[:, g * 4 * NE:(g + 1) * 4 * NE],
                        in0=h[:, :],
                        scalar=b1_t[:, 0:1],
                        in1=zcol[:, 0:1].broadcast_to([128, 4 * NE]),
                        op0=mybir.AluOpType.add,
                        op1=mybir.AluOpType.max,
                    )

            for G in range(NCHUNK // 32):
                pot = pso.tile([128, 4 * NE], F32)
                for m in range(8):
                    q = G * 8 + m
                    bi = m % 4
                    hh = m // 4
                    nc.tensor.matmul(
                        out=pot[32 * bi:32 * bi + 32, hh * 2 * NE:(hh + 1) * 2 * NE],
                        lhsT=w2_t[:, :],
                        rhs=hrelu[:, q * 2 * NE:(q + 1) * 2 * NE],
                        start=True, stop=True,
                        tile_position=(0, 32 * bi),
                    )
                nc.vector.scalar_tensor_tensor(
                    out=outsb[:, G * 4 * NE:(G + 1) * 4 * NE],
                    in0=pot[:, :],
                    scalar=1.0,
                    in1=b2_t[:, 0:1].broadcast_to([128, 4 * NE]),
                    op0=mybir.AluOpType.mult,
                    op1=mybir.AluOpType.add,
                )
            for bi in range(4):
                nc.sync.dma_start(out=out_d[bi, :, :], in_=outsb[32 * bi:32 * bi + 6, :])
    nc.compile()
    return nc


_NC = None


def _get_nc():
    global _NC
    if _NC is None:
        _NC = _build_nc()
    return _NC


def kernel(seq, W_aug, b_aug, W1, b1, W2, b2, _trace=False):
    seq = np.asarray(seq, np.float32)
    B = seq.shape[0]
    assert B == N_CORES * B_CORE, seq.shape
    A_dev, w2blk, b1t, b2t = _build_consts(W_aug, b_aug, W1, b1, W2, b2)
    nc = _get_nc()
    in_maps = []
    for i in range(N_CORES):
        z = _host_pack_z(seq[i * B_CORE:(i + 1) * B_CORE])
        in_maps.append({"z_d": z, "a_d": A_dev, "w2_d": w2blk, "b1_d": b1t, "b2_d": b2t})
    res = run_bass_kernel_spmd(nc, in_maps, core_ids=list(range(N_CORES)), trace=_trace)
    out = np.concatenate(
        [_host_unpack_out(np.asarray(r["out_d"])) for r in res.results], axis=0
    )
    if _trace:
        kernel._last_exec_time_ns = res.exec_time_ns
    return out


kernel._last_exec_time_ns = None


# revision 21
# speedup vs baseline: 1.4654x; 1.4654x over previous
"""Trainium2 Bass kernel for nn_DSTQFunction_28415503630466.

Math: the reference augments each 2-point/2-channel sequence with a pointwise
linear layer, concatenates to a 4-channel 2-point path, takes its depth-4
path signature (340 features), appends seq[:,:,-1], and applies a 2-layer MLP.
Every feature is a polynomial of degree <= 4 in the 4 raw inputs z, so the
whole pre-relu layer folds into h = A @ mono(z) + b1' where mono(z) are the 69
non-constant monomials of degree <= 4 (A computed host-side in float64).

Device (pure data parallel, 8 cores, per core B=32768 = 128 chunks x 256):
  - z arrives host-packed [128 chunks, 4*256] f32 (chunk-major, feature blocks)
  - DVE: 12 broadcast-AP tensor_mul ops build all 65 product monomials in fp16
  - DMA: relayout monomial blocks into rhs [feature-row, chunk*256+e] fp16,
    feature rows spread over partitions 0-34 / 64-97 to engage all DMA ports
  - PE: h = A.T(stationary) @ rhs per chunk, two column-packed copies of A
    (array cols 0-63 / 64-127) write PSUM rows 0-63 / 64-127 alternately
  - ACT: relu(h + b1) PSUM->SBUF fp16
  - PE: block-diag [W2;W2] (K=128) contracts pairs of chunks -> [6, N] PSUM
  - DVE: scalar_tensor_tensor adds b2 and exits PSUM->SBUF f32
  - DMA out; host reassembles (B, 3).
"""
import os
import sys

for _p in ("/opt/trn_rl_repo", "/root/.axon_site/_ro/trn_rl_repo"):
    if os.path.isdir(_p) and _p not in sys.path:
        sys.path.insert(0, _p)

import numpy as np
import concourse.bacc as bacc
import concourse.mybir as mybir
import concourse.tile as tile
from concourse.bass_utils import run_bass_kernel_spmd

F16 = mybir.dt.float16
F32 = mybir.dt.float32

N_CORES = 8
NE = 256      # elements per chunk
NCHUNK = 128  # chunks per core
B_CORE = NE * NCHUNK
N_SEGS = 4

# ---------------- host-side polynomial algebra ----------------
DEG2 = [(i, j) for i in range(4) for j in range(i, 4)]            # 10
DEG3 = [(i,) + p for i in range(4) for p in DEG2 if p[0] >= i]    # 20
DEG4 = [(i,) + p for i in range(4) for p in DEG3 if p[0] >= i]    # 35
MONOS = [(i,) for i in range(4)] + DEG2 + DEG3 + DEG4             # 69
MONO_INDEX = {m: k for k, m in enumerate(MONOS)}
NF = 69

D2_LEN = [4, 3, 2, 1]
D2_START = [0, 4, 7, 9]
D3_LEN = [10, 6, 3, 1]
D3_START = [0, 10, 16, 19]
D4_LEN = [20, 10, 4, 1]
D4_START = [0, 20, 30, 34]
D2_OFF, D3_OFF, D4_OFF = 0, 10, 30
K_SPAN = 69  # contiguous feature rows: z(0-3) deg2(4-13) deg3(14-33) deg4(34-68)


def _poly_add(a, b, sb=1.0):
    out = dict(a)
    for k, v in b.items():
        out[k] = out.get(k, 0.0) + sb * v
    return out


def _poly_scale(a, s):
    return {k: v * s for k, v in a.items()}


def _poly_mul(a, b):
    out = {}
    for ka, va in a.items():
        for kb, vb in b.items():
            k = tuple(sorted(ka + kb))
            out[k] = out.get(k, 0.0) + va * vb
    return out


def _build_A(W_aug, b_aug, W1, b1):
    """Fold augment + depth-4 signature (Chen) + W1 into (A (64,69), bias (64,))."""
    W_aug = np.asarray(W_aug, np.float64)
    b_aug = np.asarray(b_aug, np.float64)
    W1 = np.asarray(W1, np.float64)
    b1 = np.asarray(b1, np.float64)
    z = [{(i,): 1.0} for i in range(4)]

    def aug(l, d):
        s0, s1 = z[2 * l + 0], z[2 * l + 1]
        out = _poly_add(_poly_scale(s0, W_aug[d, 0]), _poly_scale(s1, W_aug[d, 1]))
        return _poly_add(out, {(): b_aug[d]})

    p = [[z[2 * l], z[2 * l + 1], aug(l, 0), aug(l, 1)] for l in range(2)]
    u = p[0]
    v = [_poly_add(p[1][c], p[0][c], -1.0) for c in range(4)]

    def sig_exp(dx):
        levels = [[dx[c] for c in range(4)]]
        for k in range(2, 5):
            levels.append(
                [_poly_scale(_poly_mul(a, dx[c]), 1.0 / k) for a in levels[-1] for c in range(4)]
            )
        return levels

    A_lv, B_lv = sig_exp(u), sig_exp(v)
    C = []
    for k in range(1, 5):
        c = [_poly_add(x, y) for x, y in zip(A_lv[k - 1], B_lv[k - 1])]
        for i in range(1, k):
            o = [_poly_mul(x, y) for x in A_lv[i - 1] for y in B_lv[k - i - 1]]
            c = [_poly_add(x, y) for x, y in zip(c, o)]
        C.append(c)
    feats = [pp for lv in C for pp in lv] + [z[1], z[3]]
    T = np.zeros((342, NF + 1))
    for r, p_ in enumerate(feats):
        for k, vv in p_.items():
            if len(k) == 0:
                T[r, NF] += vv
            else:
                T[r, MONO_INDEX[k]] += vv
    A_full = W1 @ T
    return A_full[:, :NF], A_full[:, NF] + b1


def _row_of_feature(F):
    return F


def _build_consts(W_aug, b_aug, W1, b1, W2, b2):
    A, bias1 = _build_A(W_aug, b_aug, W1, b1)
    W2 = np.asarray(W2, np.float64)
    b2 = np.asarray(b2, np.float64)
    A_dev = np.zeros((128, 64), np.float16)
    for F in range(NF):
        A_dev[_row_of_feature(F), :] = A[:, F].astype(np.float16)
    w2blk = np.zeros((128, 32), np.float16)
    w2blk[0:64, 0:3] = W2.T
    w2blk[64:128, 3:6] = W2.T
    b1t = np.zeros((128, 1), np.float32)
    b1t[0:64, 0] = bias1
    b1t[64:128, 0] = bias1
    b2t = np.zeros((128, 1), np.float32)
    for p in range(128):
        if p % 32 < 6:
            b2t[p, 0] = b2[(p % 32) % 3]
    return A_dev, w2blk, b1t, b2t


def _host_pack_z(seq_core):
    return np.ascontiguousarray(
        seq_core.reshape(NCHUNK, NE, 4).transpose(0, 2, 1)
    ).reshape(NCHUNK, 4 * NE).astype(np.float16)


def _host_unpack_out(out_d):
    """Invert the on-chip layout chain:
    out_d[bi, r, f]: f = G*4NE + hh*2NE + u -> mm2 slice q = G*8 + hh*4 + bi,
    hrelu index H = q*512 + u -> h-group g = H//(4NE), w = H%(4NE) ->
    mm1 t = 2*(w//(2NE)) + (r//3), chunk-pair cp = g*4 + t, j = w%(2NE) ->
    chunk = 2cp + j//NE, e = j%NE."""
    res = np.empty((B_CORE, 3), np.float32)
    f = np.arange(16 * NE)
    G = f // (4 * NE)
    hh = (f % (4 * NE)) // (2 * NE)
    u = f % (2 * NE)
    for bi in range(4):
        q = G * 8 + hh * 4 + bi
        H = q * (2 * NE) + u
        g = H // (4 * NE)
        w = H % (4 * NE)
        j = w % (2 * NE)
        for r in range(6):
            t = 2 * (w // (2 * NE)) + (r // 3)
            cp = g * 4 + t
            chunk = 2 * cp + j // NE
            e = j % NE
            res[chunk * NE + e, r % 3] = out_d[bi, r, :]
    return res


# ---------------- device program ----------------
def _build_nc():
    nc = bacc.Bacc(target_bir_lowering=False)
    z_d = nc.dram_tensor("z_d", [NCHUNK, 4 * NE], F16, kind="ExternalInput")
    a_d = nc.dram_tensor("a_d", [128, 64], F16, kind="ExternalInput")
    w2_d = nc.dram_tensor("w2_d", [128, 32], F16, kind="ExternalInput")
    b1_d = nc.dram_tensor("b1_d", [128, 1], F32, kind="ExternalInput")
    b2_d = nc.dram_tensor("b2_d", [128, 1], F32, kind="ExternalInput")
    out_d = nc.dram_tensor("out_d", [4, 6, 16 * NE], F32, kind="ExternalOutput")

    seg_ch = NCHUNK // N_SEGS
    with tile.TileContext(nc) as tc:
        with (
            tc.tile_pool(name="consts", bufs=1) as pc,
            tc.tile_pool(name="zp", bufs=1) as pz,
            tc.tile_pool(name="monop", bufs=1) as pm,
            tc.tile_pool(name="rhsp", bufs=1) as pr,
            tc.tile_pool(name="hrelup", bufs=1) as ph,
            tc.tile_pool(name="outp", bufs=1) as po,
            tc.tile_pool(name="psh", bufs=2, space="PSUM") as psh,
            tc.tile_pool(name="pso", bufs=2, space="PSUM") as pso,
            tc.tile_pool(name="dramp", bufs=1, space="DRAM") as pd,
        ):
            a_t = pc.tile([128, 64], F16)
            w2_t = pc.tile([128, 32], F16)
            b1_t = pc.tile([128, 1], F32)
            b2_t = pc.tile([128, 1], F32)
            nc.scalar.dma_start(out=a_t[:], in_=a_d[:])
            nc.scalar.dma_start(out=w2_t[:], in_=w2_d[:])
            nc.scalar.dma_start(out=b1_t[:], in_=b1_d[:])
            nc.scalar.dma_start(out=b2_t[:], in_=b2_d[:])

            zcol = pc.tile([128, 1], F32)
            nc.vector.memset(zcol[:, :], 0.0)
            feat = pm.tile([NCHUNK, 69 * NE], F16)   # blocks: z(4) deg2(10) deg3(20) deg4(35)
            zh = feat[:, 0:4 * NE]
            rhs = pr.tile([128, NCHUNK * NE], F16)
            hrelu = ph.tile([128, (NCHUNK // 2) * NE], F16)
            outsb = po.tile([128, 16 * NE], F32)
            bounce = pd.tile([NCHUNK, 69 * NE], F16)

            def blk(t, b0, n):
                return t[:, b0 * NE:(b0 + n) * NE].rearrange("p (f e) -> p f e", e=NE)

            CG = NCHUNK // 4  # bounce in 4 chunk-groups so mm starts early

            def bounce_pair(F0, n, cg):
                """feat blocks [F0, F0+n) x chunk-group cg -> DRAM -> rhs rows.
                One dump (<=128 contiguous descriptors), then strided (f,c,e)
                gathers capped at 16 rows x 32 chunks = 512 descriptors per
                dma_start so no HWDGE descriptor ring overflows (a single
                35x128 = 4480-descriptor gather kills the device)."""
                cs = slice(cg * CG, (cg + 1) * CG)
                nc.sync.dma_start(
                    out=bounce[cs, F0 * NE:(F0 + n) * NE],
                    in_=feat[cs, F0 * NE:(F0 + n) * NE],
                )
                for f0 in range(F0, F0 + n, 16):
                    nf = min(16, F0 + n - f0)
                    src = bounce[cs, f0 * NE:(f0 + nf) * NE]
                    src = src.rearrange("c (f e) -> c f e", e=NE).transpose([1, 0, 2])
                    nc.sync.dma_start(
                        out=rhs[f0:f0 + nf, cg * CG * NE:(cg + 1) * CG * NE].rearrange(
                            "f (c e) -> f c e", e=NE
                        ),
                        in_=src,
                    )

            # products: full 128-partition ops (DVE time = free-size; never
            # partition-slice); feat blocks: z at 0..3, deg-k groups offset +4
            nc.sync.dma_start(out=zh[:, :], in_=z_d[:, :])
            for i in range(4):
                n = D2_LEN[i]
                nc.vector.tensor_mul(
                    out=blk(feat, 4 + D2_OFF + D2_START[i], n),
                    in0=zh[:, i * NE:(i + 1) * NE].unsqueeze(1).broadcast_to([NCHUNK, n, NE]),
                    in1=blk(feat, i, n),
                )
            bounce_pair(0, 14, 0)   # z + deg2 rows, chunk-group 0
            for i in (3, 2, 1, 0):
                n = D3_LEN[i]
                nc.vector.tensor_mul(
                    out=blk(feat, 4 + D3_OFF + D3_START[i], n),
                    in0=zh[:, i * NE:(i + 1) * NE].unsqueeze(1).broadcast_to([NCHUNK, n, NE]),
                    in1=blk(feat, 4 + D2_OFF + D2_START[i], n),
                )
            bounce_pair(14, 20, 0)  # deg3 rows, cg0
            bounce_pair(0, 14, 1)
            for i in (3, 2, 1, 0):
                n4 = D4_LEN[i]
                nc.vector.tensor_mul(
                    out=blk(feat, 4 + D4_OFF + D4_START[i], n4),
                    in0=zh[:, i * NE:(i + 1) * NE].unsqueeze(1).broadcast_to([NCHUNK, n4, NE]),
                    in1=blk(feat, 4 + D3_OFF + D3_START[i], n4),
                )
            bounce_pair(34, 35, 0)  # deg4 rows, cg0 -> mm chunk-group 0 ready
            bounce_pair(14, 20, 1)
            bounce_pair(34, 35, 1)
            bounce_pair(0, 34, 2)
            bounce_pair(34, 35, 2)
            bounce_pair(0, 34, 3)
            bounce_pair(34, 35, 3)

            for g in range(NCHUNK // 8):
                h = psh.tile([128, 4 * NE], F32)
                for t in range(4):  # chunk-pair (2t, 2t+1) -> one N=512 matmul
                    cp = g * 4 + t
                    half = slice(0, 64) if (t % 2 == 0) else slice(64, 128)
                    fr = (t // 2) * 2 * NE
                    nc.tensor.matmul(
                        out=h[half, fr:fr + 2 * NE],
                        lhsT=a_t[0:K_SPAN, :],
                        rhs=rhs[0:K_SPAN, cp * 2 * NE:(cp + 1) * 2 * NE],
                        start=True, stop=True,
                    )
                if g % 8 >= 3:  # balance: 10 groups on ACT (late), 6 on DVE (early)
                    nc.scalar.activation(
                        out=hrelu[:, g * 4 * NE:(g + 1) * 4 * NE],
                        in_=h[:, :],
                        func=mybir.ActivationFunctionType.Relu,
                        bias=b1_t[:, 0:1],
                        scale=1.0,
                    )
                else:
                    nc.vector.scalar_tensor_tensor(
                        out=hrelu[:, g * 4 * NE:(g + 1) * 4 * NE],
                        in0=h[:, :],
                        scalar=b1_t[:, 0:1],
                        in1=zcol[:, 0:1].broadcast_to([128, 4 * NE]),
                        op0=mybir.AluOpType.add,
                        op1=mybir.AluOpType.max,
                    )

            for G in range(NCHUNK // 32):
                pot = pso.tile([128, 4 * NE], F32)
                for m in range(8):
                    q = G * 8 + m
                    bi = m % 4
                    hh = m // 4
                    nc.tensor.matmul(
                        out=pot[32 * bi:32 * bi + 32, hh * 2 * NE:(hh + 1) * 2 * NE],
                        lhsT=w2_t[:, :],
                        rhs=hrelu[:, q * 2 * NE:(q + 1) * 2 * NE],
                        start=True, stop=True,
                        tile_position=(0, 32 * bi),
                    )
                nc.vector.scalar_tensor_tensor(
                    out=outsb[:, G * 4 * NE:(G + 1) * 4 * NE],
                    in0=pot[:, :],
                    scalar=1.0,
                    in1=b2_t[:, 0:1].broadcast_to([128, 4 * NE]),
                    op0=mybir.AluOpType.mult,
                    op1=mybir.AluOpType.add,
                )
            for bi in range(4):
                nc.sync.dma_start(out=out_d[bi, :, :], in_=outsb[32 * bi:32 * bi + 6, :])
    nc.compile()
    return nc


_NC = None


def _get_nc():
    global _NC
    if _NC is None:
        _NC = _build_nc()
    return _NC


def kernel(seq, W_aug, b_aug, W1, b1, W2, b2, _trace=False):
    seq = np.asarray(seq, np.float32)
    B = seq.shape[0]
    assert B == N_CORES * B_CORE, seq.shape
    A_dev, w2blk, b1t, b2t = _build_consts(W_aug, b_aug, W1, b1, W2, b2)
    nc = _get_nc()
    in_maps = []
    for i in range(N_CORES):
        z = _host_pack_z(seq[i * B_CORE:(i + 1) * B_CORE])
        in_maps.append({"z_d": z, "a_d": A_dev, "w2_d": w2blk, "b1_d": b1t, "b2_d": b2t})
    res = run_bass_kernel_spmd(nc, in_maps, core_ids=list(range(N_CORES)), trace=_trace)
    out = np.concatenate(
        [_host_unpack_out(np.asarray(r["out_d"])) for r in res.results], axis=0
    )
    if _trace:
        kernel._last_exec_time_ns = res.exec_time_ns
    return out


kernel._last_exec_time_ns = None


# revision 22
# speedup vs baseline: 1.5081x; 1.0291x over previous
"""Trainium2 Bass kernel for nn_DSTQFunction_28415503630466.

Math: the reference augments each 2-point/2-channel sequence with a pointwise
linear layer, concatenates to a 4-channel 2-point path, takes its depth-4
path signature (340 features), appends seq[:,:,-1], and applies a 2-layer MLP.
Every feature is a polynomial of degree <= 4 in the 4 raw inputs z, so the
whole pre-relu layer folds into h = A @ mono(z) + b1' where mono(z) are the 69
non-constant monomials of degree <= 4 (A computed host-side in float64).

Device (pure data parallel, 8 cores, per core B=32768 = 128 chunks x 256):
  - z arrives host-packed [128 chunks, 4*256] f32 (chunk-major, feature blocks)
  - DVE: 12 broadcast-AP tensor_mul ops build all 65 product monomials in fp16
  - DMA: relayout monomial blocks into rhs [feature-row, chunk*256+e] fp16,
    feature rows spread over partitions 0-34 / 64-97 to engage all DMA ports
  - PE: h = A.T(stationary) @ rhs per chunk, two column-packed copies of A
    (array cols 0-63 / 64-127) write PSUM rows 0-63 / 64-127 alternately
  - ACT: relu(h + b1) PSUM->SBUF fp16
  - PE: block-diag [W2;W2] (K=128) contracts pairs of chunks -> [6, N] PSUM
  - DVE: scalar_tensor_tensor adds b2 and exits PSUM->SBUF f32
  - DMA out; host reassembles (B, 3).
"""
import os
import sys

for _p in ("/opt/trn_rl_repo", "/root/.axon_site/_ro/trn_rl_repo"):
    if os.path.isdir(_p) and _p not in sys.path:
        sys.path.insert(0, _p)

import numpy as np
import concourse.bacc as bacc
import concourse.mybir as mybir
import concourse.tile as tile
from concourse.bass_utils import run_bass_kernel_spmd

F16 = mybir.dt.float16
F32 = mybir.dt.float32

N_CORES = 8
NE = 256      # elements per chunk
NCHUNK = 128  # chunks per core
B_CORE = NE * NCHUNK
N_SEGS = 4

# ---------------- host-side polynomial algebra ----------------
DEG2 = [(i, j) for i in range(4) for j in range(i, 4)]            # 10
DEG3 = [(i,) + p for i in range(4) for p in DEG2 if p[0] >= i]    # 20
DEG4 = [(i,) + p for i in range(4) for p in DEG3 if p[0] >= i]    # 35
MONOS = [(i,) for i in range(4)] + DEG2 + DEG3 + DEG4             # 69
MONO_INDEX = {m: k for k, m in enumerate(MONOS)}
NF = 69

D2_LEN = [4, 3, 2, 1]
D2_START = [0, 4, 7, 9]
D3_LEN = [10, 6, 3, 1]
D3_START = [0, 10, 16, 19]
D4_LEN = [20, 10, 4, 1]
D4_START = [0, 20, 30, 34]
D2_OFF, D3_OFF, D4_OFF = 0, 10, 30
K_SPAN = 69  # contiguous feature rows: z(0-3) deg2(4-13) deg3(14-33) deg4(34-68)


def _poly_add(a, b, sb=1.0):
    out = dict(a)
    for k, v in b.items():
        out[k] = out.get(k, 0.0) + sb * v
    return out


def _poly_scale(a, s):
    return {k: v * s for k, v in a.items()}


def _poly_mul(a, b):
    out = {}
    for ka, va in a.items():
        for kb, vb in b.items():
            k = tuple(sorted(ka + kb))
            out[k] = out.get(k, 0.0) + va * vb
    return out


def _build_A(W_aug, b_aug, W1, b1):
    """Fold augment + depth-4 signature (Chen) + W1 into (A (64,69), bias (64,))."""
    W_aug = np.asarray(W_aug, np.float64)
    b_aug = np.asarray(b_aug, np.float64)
    W1 = np.asarray(W1, np.float64)
    b1 = np.asarray(b1, np.float64)
    z = [{(i,): 1.0} for i in range(4)]

    def aug(l, d):
        s0, s1 = z[2 * l + 0], z[2 * l + 1]
        out = _poly_add(_poly_scale(s0, W_aug[d, 0]), _poly_scale(s1, W_aug[d, 1]))
        return _poly_add(out, {(): b_aug[d]})

    p = [[z[2 * l], z[2 * l + 1], aug(l, 0), aug(l, 1)] for l in range(2)]
    u = p[0]
    v = [_poly_add(p[1][c], p[0][c], -1.0) for c in range(4)]

    def sig_exp(dx):
        levels = [[dx[c] for c in range(4)]]
        for k in range(2, 5):
            levels.append(
                [_poly_scale(_poly_mul(a, dx[c]), 1.0 / k) for a in levels[-1] for c in range(4)]
            )
        return levels

    A_lv, B_lv = sig_exp(u), sig_exp(v)
    C = []
    for k in range(1, 5):
        c = [_poly_add(x, y) for x, y in zip(A_lv[k - 1], B_lv[k - 1])]
        for i in range(1, k):
            o = [_poly_mul(x, y) for x in A_lv[i - 1] for y in B_lv[k - i - 1]]
            c = [_poly_add(x, y) for x, y in zip(c, o)]
        C.append(c)
    feats = [pp for lv in C for pp in lv] + [z[1], z[3]]
    T = np.zeros((342, NF + 1))
    for r, p_ in enumerate(feats):
        for k, vv in p_.items():
            if len(k) == 0:
                T[r, NF] += vv
            else:
                T[r, MONO_INDEX[k]] += vv
    A_full = W1 @ T
    return A_full[:, :NF], A_full[:, NF] + b1


def _row_of_feature(F):
    return F


def _build_consts(W_aug, b_aug, W1, b1, W2, b2):
    A, bias1 = _build_A(W_aug, b_aug, W1, b1)
    W2 = np.asarray(W2, np.float64)
    b2 = np.asarray(b2, np.float64)
    A_dev = np.zeros((128, 64), np.float16)
    for F in range(NF):
        A_dev[_row_of_feature(F), :] = A[:, F].astype(np.float16)
    w2blk = np.zeros((128, 32), np.float16)
    w2blk[0:64, 0:3] = W2.T
    w2blk[64:128, 3:6] = W2.T
    b1t = np.zeros((128, 1), np.float32)
    b1t[0:64, 0] = bias1
    b1t[64:128, 0] = bias1
    b2t = np.zeros((128, 1), np.float32)
    for p in range(128):
        if p % 32 < 6:
            b2t[p, 0] = b2[(p % 32) % 3]
    return A_dev, w2blk, b1t, b2t


def _host_pack_z(seq_core):
    return np.ascontiguousarray(
        seq_core.reshape(NCHUNK, NE, 4).transpose(0, 2, 1)
    ).reshape(NCHUNK, 4 * NE).astype(np.float16)


def _host_unpack_out(out_d):
    """Invert the on-chip layout chain:
    out_d[bi, r, f]: f = G*4NE + hh*2NE + u -> mm2 slice q = G*8 + hh*4 + bi,
    hrelu index H = q*512 + u -> h-group g = H//(4NE), w = H%(4NE) ->
    mm1 t = 2*(w//(2NE)) + (r//3), chunk-pair cp = g*4 + t, j = w%(2NE) ->
    chunk = 2cp + j//NE, e = j%NE."""
    res = np.empty((B_CORE, 3), np.float32)
    f = np.arange(16 * NE)
    G = f // (4 * NE)
    hh = (f % (4 * NE)) // (2 * NE)
    u = f % (2 * NE)
    for bi in range(4):
        q = G * 8 + hh * 4 + bi
        H = q * (2 * NE) + u
        g = H // (4 * NE)
        w = H % (4 * NE)
        j = w % (2 * NE)
        for r in range(6):
            t = 2 * (w // (2 * NE)) + (r // 3)
            cp = g * 4 + t
            chunk = 2 * cp + j // NE
            e = j % NE
            res[chunk * NE + e, r % 3] = out_d[bi, r, :]
    return res


# ---------------- device program ----------------
def _build_nc():
    nc = bacc.Bacc(target_bir_lowering=False)
    z_d = nc.dram_tensor("z_d", [NCHUNK, 4 * NE], F16, kind="ExternalInput")
    a_d = nc.dram_tensor("a_d", [128, 64], F16, kind="ExternalInput")
    w2_d = nc.dram_tensor("w2_d", [128, 32], F16, kind="ExternalInput")
    b1_d = nc.dram_tensor("b1_d", [128, 1], F32, kind="ExternalInput")
    b2_d = nc.dram_tensor("b2_d", [128, 1], F32, kind="ExternalInput")
    out_d = nc.dram_tensor("out_d", [4, 6, 16 * NE], F32, kind="ExternalOutput")

    seg_ch = NCHUNK // N_SEGS
    with tile.TileContext(nc) as tc:
        with (
            tc.tile_pool(name="consts", bufs=1) as pc,
            tc.tile_pool(name="zp", bufs=1) as pz,
            tc.tile_pool(name="monop", bufs=1) as pm,
            tc.tile_pool(name="rhsp", bufs=1) as pr,
            tc.tile_pool(name="hrelup", bufs=1) as ph,
            tc.tile_pool(name="outp", bufs=1) as po,
            tc.tile_pool(name="psh", bufs=2, space="PSUM") as psh,
            tc.tile_pool(name="pso", bufs=2, space="PSUM") as pso,
            tc.tile_pool(name="dramp", bufs=1, space="DRAM") as pd,
        ):
            a_t = pc.tile([128, 64], F16)
            w2_t = pc.tile([128, 32], F16)
            b1_t = pc.tile([128, 1], F32)
            b2_t = pc.tile([128, 1], F32)
            nc.scalar.dma_start(out=a_t[:], in_=a_d[:])
            nc.scalar.dma_start(out=w2_t[:], in_=w2_d[:])
            nc.scalar.dma_start(out=b1_t[:], in_=b1_d[:])
            nc.scalar.dma_start(out=b2_t[:], in_=b2_d[:])

            zcol = pc.tile([128, 1], F32)
            nc.vector.memset(zcol[:, :], 0.0)
            feat = pm.tile([NCHUNK, 69 * NE], F16)   # blocks: z(4) deg2(10) deg3(20) deg4(35)
            zh = feat[:, 0:4 * NE]
            rhs = pr.tile([128, NCHUNK * NE], F16)
            hrelu = ph.tile([128, (NCHUNK // 2) * NE], F16)
            outsb = po.tile([128, 16 * NE], F32)
            bounce = pd.tile([NCHUNK, 69 * NE], F16)

            def blk(t, b0, n):
                return t[:, b0 * NE:(b0 + n) * NE].rearrange("p (f e) -> p f e", e=NE)

            CG = NCHUNK // 4  # bounce in 4 chunk-groups so mm starts early

            def bounce_pair(F0, n, cg):
                """feat blocks [F0, F0+n) x chunk-group cg -> DRAM -> rhs rows.
                One dump (<=128 contiguous descriptors), then strided (f,c,e)
                gathers capped at 16 rows x 32 chunks = 512 descriptors per
                dma_start so no HWDGE descriptor ring overflows (a single
                35x128 = 4480-descriptor gather kills the device)."""
                cs = slice(cg * CG, (cg + 1) * CG)
                nc.sync.dma_start(
                    out=bounce[cs, F0 * NE:(F0 + n) * NE],
                    in_=feat[cs, F0 * NE:(F0 + n) * NE],
                )
                for f0 in range(F0, F0 + n, 16):
                    nf = min(16, F0 + n - f0)
                    src = bounce[cs, f0 * NE:(f0 + nf) * NE]
                    src = src.rearrange("c (f e) -> c f e", e=NE).transpose([1, 0, 2])
                    nc.sync.dma_start(
                        out=rhs[f0:f0 + nf, cg * CG * NE:(cg + 1) * CG * NE].rearrange(
                            "f (c e) -> f c e", e=NE
                        ),
                        in_=src,
                    )

            # products: full 128-partition ops (DVE time = free-size; never
            # partition-slice); feat blocks: z at 0..3, deg-k groups offset +4
            nc.sync.dma_start(out=zh[:, :], in_=z_d[:, :])
            for i in range(4):
                n = D2_LEN[i]
                nc.vector.tensor_mul(
                    out=blk(feat, 4 + D2_OFF + D2_START[i], n),
                    in0=zh[:, i * NE:(i + 1) * NE].unsqueeze(1).broadcast_to([NCHUNK, n, NE]),
                    in1=blk(feat, i, n),
                )
            def z_gather(cg):
                # rhs rows 0..3 gathered straight from the DRAM input (f,c,e)
                cs = slice(cg * CG, (cg + 1) * CG)
                src = z_d[cs, :].rearrange("c (f e) -> c f e", e=NE).transpose([1, 0, 2])
                nc.sync.dma_start(
                    out=rhs[0:4, cg * CG * NE:(cg + 1) * CG * NE].rearrange(
                        "f (c e) -> f c e", e=NE
                    ),
                    in_=src,
                )

            for cg in range(4):
                z_gather(cg)
            bounce_pair(4, 10, 0)   # deg2 rows, chunk-group 0
            for i in (3, 2, 1, 0):
                n = D3_LEN[i]
                nc.vector.tensor_mul(
                    out=blk(feat, 4 + D3_OFF + D3_START[i], n),
                    in0=zh[:, i * NE:(i + 1) * NE].unsqueeze(1).broadcast_to([NCHUNK, n, NE]),
                    in1=blk(feat, 4 + D2_OFF + D2_START[i], n),
                )
            bounce_pair(14, 20, 0)  # deg3 rows, cg0
            bounce_pair(4, 10, 1)
            for i in (3, 2, 1, 0):
                n4 = D4_LEN[i]
                nc.vector.tensor_mul(
                    out=blk(feat, 4 + D4_OFF + D4_START[i], n4),
                    in0=zh[:, i * NE:(i + 1) * NE].unsqueeze(1).broadcast_to([NCHUNK, n4, NE]),
                    in1=blk(feat, 4 + D3_OFF + D3_START[i], n4),
                )
            bounce_pair(34, 35, 0)  # deg4 rows, cg0 -> mm chunk-group 0 ready
            bounce_pair(14, 20, 1)
            bounce_pair(34, 35, 1)
            bounce_pair(4, 30, 2)
            bounce_pair(34, 35, 2)
            bounce_pair(4, 30, 3)
            bounce_pair(34, 35, 3)

            for g in range(NCHUNK // 8):
                h = psh.tile([128, 4 * NE], F32)
                for t in range(4):  # chunk-pair (2t, 2t+1) -> one N=512 matmul
                    cp = g * 4 + t
                    half = slice(0, 64) if (t % 2 == 0) else slice(64, 128)
                    fr = (t // 2) * 2 * NE
                    nc.tensor.matmul(
                        out=h[half, fr:fr + 2 * NE],
                        lhsT=a_t[0:K_SPAN, :],
                        rhs=rhs[0:K_SPAN, cp * 2 * NE:(cp + 1) * 2 * NE],
                        start=True, stop=True,
                    )
                if g % 8 >= 3:  # balance: 10 groups on ACT (late), 6 on DVE (early)
                    nc.scalar.activation(
                        out=hrelu[:, g * 4 * NE:(g + 1) * 4 * NE],
                        in_=h[:, :],
                        func=mybir.ActivationFunctionType.Relu,
                        bias=b1_t[:, 0:1],
                        scale=1.0,
                    )
                else:
                    nc.vector.scalar_tensor_tensor(
                        out=hrelu[:, g * 4 * NE:(g + 1) * 4 * NE],
                        in0=h[:, :],
                        scalar=b1_t[:, 0:1],
                        in1=zcol[:, 0:1].broadcast_to([128, 4 * NE]),
                        op0=mybir.AluOpType.add,
                        op1=mybir.AluOpType.max,
                    )

            for G in range(NCHUNK // 32):
                pot = pso.tile([128, 4 * NE], F32)
                for m in range(8):
                    q = G * 8 + m
                    bi = m % 4
                    hh = m // 4
                    nc.tensor.matmul(
                        out=pot[32 * bi:32 * bi + 32, hh * 2 * NE:(hh + 1) * 2 * NE],
                        lhsT=w2_t[:, :],
                        rhs=hrelu[:, q * 2 * NE:(q + 1) * 2 * NE],
                        start=True, stop=True,
                        tile_position=(0, 32 * bi),
                    )
                nc.vector.scalar_tensor_tensor(
                    out=outsb[:, G * 4 * NE:(G + 1) * 4 * NE],
                    in0=pot[:, :],
                    scalar=1.0,
                    in1=b2_t[:, 0:1].broadcast_to([128, 4 * NE]),
                    op0=mybir.AluOpType.mult,
                    op1=mybir.AluOpType.add,
                )
            for bi in range(4):
                nc.sync.dma_start(out=out_d[bi, :, :], in_=outsb[32 * bi:32 * bi + 6, :])
    nc.compile()
    return nc


_NC = None


def _get_nc():
    global _NC
    if _NC is None:
        _NC = _build_nc()
    return _NC


def kernel(seq, W_aug, b_aug, W1, b1, W2, b2, _trace=False):
    seq = np.asarray(seq, np.float32)
    B = seq.shape[0]
    assert B == N_CORES * B_CORE, seq.shape
    A_dev, w2blk, b1t, b2t = _build_consts(W_aug, b_aug, W1, b1, W2, b2)
    nc = _get_nc()
    in_maps = []
    for i in range(N_CORES):
        z = _host_pack_z(seq[i * B_CORE:(i + 1) * B_CORE])
        in_maps.append({"z_d": z, "a_d": A_dev, "w2_d": w2blk, "b1_d": b1t, "b2_d": b2t})
    res = run_bass_kernel_spmd(nc, in_maps, core_ids=list(range(N_CORES)), trace=_trace)
    out = np.concatenate(
        [_host_unpack_out(np.asarray(r["out_d"])) for r in res.results], axis=0
    )
    if _trace:
        kernel._last_exec_time_ns = res.exec_time_ns
    return out


kernel._last_exec_time_ns = None
